# revision 5
# baseline (speedup 1.0000x reference)
"""Trainium2 Bass kernel for nn_Attn (Bahdanau-style attention scores).

Reference computation:
    energy[s,b,:] = W @ enc[s,b,:] + bias          [S,B,H]
    scores[b,s]   = hidden[0,b,:] . energy[s,b,:]  [B,S]
    out           = softmax(scores, axis=-1)[:,None,:]

Key rewrite: scores[b,s] = (W^T hidden_b) . enc[s,b,:] + hidden_b . bias.
The second term is constant in s, so it is invariant under softmax and is
dropped entirely.  v_b = W^T hidden_b is a tiny [B, 2H] matvec done on the
tensor engine; the S*B*2H dot-product sweep is done by the vector engine
with fused tensor_tensor_reduce while DMA streams enc at full HBM bandwidth.

Sharding: data-parallel over batch B (4 batch rows per core, 8 cores).
Each core receives enc[:, b0:b0+4, :] (64 MiB), hidden^T slice and W.
"""

import numpy as np

# Problem sizes (hardcoded per harness contract).
H = 1024          # hidden size
K = 2 * H         # 2H = contraction dim of W
S = 2048          # encoder sequence length
B = 32            # batch
N_CORES = 8
BPC = B // N_CORES  # batch rows per core = 4

ST = 128          # s-tile (partition dim)
NST = S // ST     # 16 s-tiles
KC = 512          # psum free chunk for the v matmul
NKC = K // KC     # 4
HC = 128          # h chunk (matmul contraction tile)
NHC = H // HC     # 8

_CACHE = {}


def _emit(ctx, tc, enc, hidT, w, out):
    """Emit the per-core program.

    enc : DRAM [S, BPC, K]  fp32
    hidT: DRAM [128, NHC*BPC] fp32, layout [p][c][b] for h = c*128 + p
    w   : DRAM [H, K] fp32
    out : DRAM [BPC, S] fp32  (softmax probabilities)
    """
    import concourse.bass as bass
    from concourse import mybir
    from concourse.bass_isa import ReduceOp
    from concourse.masks import make_identity

    nc = tc.nc
    f32 = mybir.dt.float32

    singles = ctx.enter_context(tc.tile_pool(name="singles", bufs=1))
    wpool = ctx.enter_context(tc.tile_pool(name="wpool", bufs=2))
    encpool = ctx.enter_context(tc.tile_pool(name="encp", bufs=3))
    vpsum = ctx.enter_context(tc.tile_pool(name="vpsum", bufs=1, space="PSUM"))
    tpsum = ctx.enter_context(tc.tile_pool(name="tpsum", bufs=1, space="PSUM"))
    dram = ctx.enter_context(tc.tile_pool(name="dram", bufs=1, space="DRAM"))
    small = ctx.enter_context(tc.tile_pool(name="small", bufs=2))

    # ---- load hidden^T (tiny) -------------------------------------------
    hid_sb = singles.tile([128, NHC * BPC], f32)
    nc.sync.dma_start(out=hid_sb, in_=hidT)

    # ---- v = W^T h : PE matmul, contraction over h ----------------------
    # out[m=b, n=k] = sum_h hidT[h, b] * W[h, k], accumulated over NHC chunks
    v_ps = [
        vpsum.tile([BPC, KC], f32, name=f"v_ps{i}", tag=f"v_ps{i}")
        for i in range(NKC)
    ]
    for c in range(NHC):
        w_sb = wpool.tile([128, K], f32)
        nc.sync.dma_start(out=w_sb, in_=w[c * HC:(c + 1) * HC, :])
        for kc in range(NKC):
            nc.tensor.matmul(
                v_ps[kc][:, :],
                lhsT=hid_sb[:, c * BPC:(c + 1) * BPC],
                rhs=w_sb[:, kc * KC:(kc + 1) * KC],
                start=(c == 0),
                stop=(c == NHC - 1),
            )

    v_sb = small.tile([BPC, K], f32)
    for kc in range(NKC):
        nc.scalar.copy(out=v_sb[:, kc * KC:(kc + 1) * KC], in_=v_ps[kc][:, :])

    # ---- broadcast v to all 128 partitions via a DRAM bounce ------------
    v_dr = dram.tile([BPC, K], f32)
    nc.sync.dma_start(out=v_dr[:, :], in_=v_sb[:, :])
    v_ap = v_dr[:, :]
    v_bcast_src = bass.AP(
        tensor=v_ap.tensor, offset=v_ap.offset, ap=[[0, 128]] + list(v_ap.ap)
    )
    v_bc = singles.tile([128, BPC, K], f32)
    nc.gpsimd.dma_start(out=v_bc, in_=v_bcast_src)

    # ---- main sweep: scores[s,b] = enc[s,b,:] . v_b ---------------------
    # DVE does the elementwise multiply; ScalarE (activation Copy with
    # accum_out) does the free-dim sum, so the two passes run on separate
    # engines and both stay under the DMA streaming time.
    scores = singles.tile([128, BPC, NST], f32)
    prodpool = ctx.enter_context(tc.tile_pool(name="prodp", bufs=3))
    for st in range(NST):
        enc_sb = encpool.tile([128, BPC, K], f32)
        eng = nc.sync if (st % 2 == 0) else nc.scalar
        eng.dma_start(out=enc_sb, in_=enc[st * ST:(st + 1) * ST, :, :])
        for b in range(BPC):
            prod = prodpool.tile([128, K], f32, name="prod", tag="prod")
            nc.vector.tensor_mul(prod, enc_sb[:, b, :], v_bc[:, b, :])
            nc.scalar.activation(
                out=prod,
                in_=prod,
                func=mybir.ActivationFunctionType.Copy,
                bias=0.0,
                scale=1.0,
                accum_out=scores[:, b, st:st + 1],
            )

    # ---- softmax over s (s is split: 128 partitions x NST cols) ---------
    m = small.tile([128, BPC], f32)
    nc.vector.tensor_reduce(
        out=m, in_=scores, axis=mybir.AxisListType.X, op=mybir.AluOpType.max
    )
    mm = small.tile([128, BPC], f32)
    nc.gpsimd.partition_all_reduce(mm, m, 128, ReduceOp.max)
    nm = small.tile([128, BPC], f32)
    nc.vector.tensor_scalar_mul(nm, mm, -1.0)

    e = singles.tile([128, BPC, NST], f32)
    r = small.tile([128, BPC], f32)
    for b in range(BPC):
        nc.scalar.activation(
            out=e[:, b, :],
            in_=scores[:, b, :],
            func=mybir.ActivationFunctionType.Exp,
            bias=nm[:, b:b + 1],
            scale=1.0,
            accum_out=r[:, b:b + 1],
        )
    rs = small.tile([128, BPC], f32)
    nc.gpsimd.partition_all_reduce(rs, r, 128, ReduceOp.add)
    inv = small.tile([128, BPC], f32)
    nc.vector.reciprocal(inv, rs)

    attn = singles.tile([128, BPC * NST], f32)
    attn3 = attn.rearrange("p (b t) -> p b t", b=BPC)
    for b in range(BPC):
        nc.vector.tensor_scalar_mul(attn3[:, b, :], e[:, b, :], inv[:, b:b + 1])

    # ---- transpose [128, 64] -> [64, 128] and store ---------------------
    ident = singles.tile([128, 128], f32)
    make_identity(nc, ident)
    att_ps = tpsum.tile([BPC * NST, 128], f32)
    nc.tensor.transpose(att_ps[:, :], attn[:, :], ident[:, :])
    att_sb = small.tile([BPC * NST, 128], f32)
    nc.scalar.copy(out=att_sb, in_=att_ps[:, :])
    nc.sync.dma_start(
        out=out.rearrange("b (t s) -> b t s", s=ST), in_=att_sb
    )


def _build():
    if "nc" in _CACHE:
        return _CACHE["nc"]
    from contextlib import ExitStack

    import concourse.bacc as bacc
    import concourse.tile as tile
    from concourse import mybir

    nc = bacc.Bacc(
        "TRN2", target_bir_lowering=False, debug=False, num_devices=N_CORES
    )
    enc_d = nc.dram_tensor("enc", [S, BPC, K], mybir.dt.float32, kind="ExternalInput")
    hid_d = nc.dram_tensor(
        "hidT", [128, NHC * BPC], mybir.dt.float32, kind="ExternalInput"
    )
    w_d = nc.dram_tensor("w", [H, K], mybir.dt.float32, kind="ExternalInput")
    out_d = nc.dram_tensor(
        "attn_out", [BPC, S], mybir.dt.float32, kind="ExternalOutput"
    )

    with tile.TileContext(nc) as tc:
        with ExitStack() as ctx:
            _emit(ctx, tc, enc_d.ap(), hid_d.ap(), w_d.ap(), out_d.ap())
    nc.compile()
    _CACHE["nc"] = nc
    return nc


def _make_in_maps(hidden, encoder_outputs, W):
    in_maps = []
    w = np.ascontiguousarray(W, dtype=np.float32)
    for i in range(N_CORES):
        b0 = i * BPC
        # hidT layout [p][c][b] with h = c*128 + p
        hid = hidden[0, b0:b0 + BPC, :]                    # [BPC, H]
        hidT = np.ascontiguousarray(
            hid.T.reshape(NHC, 128, BPC).transpose(1, 0, 2).reshape(128, NHC * BPC),
            dtype=np.float32,
        )
        enc = np.ascontiguousarray(
            encoder_outputs[:, b0:b0 + BPC, :], dtype=np.float32
        )
        in_maps.append({"enc": enc, "hidT": hidT, "w": w})
    return in_maps


def kernel(hidden, encoder_outputs, W, b):
    from concourse import bass_utils

    nc = _build()
    in_maps = _make_in_maps(
        np.asarray(hidden), np.asarray(encoder_outputs), np.asarray(W)
    )
    res = bass_utils.run_bass_kernel_spmd(
        nc, in_maps, core_ids=list(range(N_CORES))
    )
    out = np.concatenate(
        [res.results[i]["attn_out"] for i in range(N_CORES)], axis=0
    )  # [B, S]
    return out[:, None, :].astype(np.float32)


# revision 9
# speedup vs baseline: 1.1323x; 1.1323x over previous
"""Trainium2 Bass kernel for nn_Attn (Bahdanau-style attention scores).

Reference computation:
    energy[s,b,:] = W @ enc[s,b,:] + bias          [S,B,H]
    scores[b,s]   = hidden[0,b,:] . energy[s,b,:]  [B,S]
    out           = softmax(scores, axis=-1)[:,None,:]

Key rewrite: scores[b,s] = (W^T hidden_b) . enc[s,b,:] + hidden_b . bias.
The second term is constant in s, so it is invariant under softmax and is
dropped entirely.  v_b = W^T hidden_b is a tiny [B, 2H] matvec done on the
tensor engine; the S*B*2H dot-product sweep is done by the vector engine
(elementwise multiply) + scalar engine (activation-Copy with accum_out for
the free-dim sum) while DMA streams enc at full HBM bandwidth.

Sharding: data-parallel over batch B (4 batch rows per core, 8 cores).
Each core receives enc[:, b0:b0+4, :] (64 MiB), hidden^T slice and W.
"""

import numpy as np

# Problem sizes (hardcoded per harness contract).
H = 1024          # hidden size
K = 2 * H         # 2H = contraction dim of W
S = 2048          # encoder sequence length
B = 32            # batch
N_CORES = 8
BPC = B // N_CORES  # batch rows per core = 4

ST = 128          # s-tile (partition dim)
NST = S // ST     # 16 s-tiles
KC = 512          # psum free chunk for the v matmul
NKC = K // KC     # 4
HC = 128          # h chunk (matmul contraction tile)
NHC = H // HC     # 8
BGRP = 2          # batch rows per enc DMA tile

_CACHE = {}


def _emit(ctx, tc, enc, hidT, w, out):
    """Emit the per-core program.

    enc : DRAM [S, BPC, K]  fp32
    hidT: DRAM [128, NHC*BPC] fp32, layout [p][c][b] for h = c*128 + p
    w   : DRAM [H, K] fp32
    out : DRAM [BPC, S] fp32  (softmax probabilities)
    """
    from concourse import mybir
    from concourse.bass_isa import ReduceOp
    from concourse.masks import make_identity

    nc = tc.nc
    f32 = mybir.dt.float32

    singles = ctx.enter_context(tc.tile_pool(name="singles", bufs=1))
    wpool = ctx.enter_context(tc.tile_pool(name="wpool", bufs=2))
    encpool = ctx.enter_context(tc.tile_pool(name="encp", bufs=6))
    prodpool = ctx.enter_context(tc.tile_pool(name="prodp", bufs=2))
    vpsum = ctx.enter_context(tc.tile_pool(name="vpsum", bufs=1, space="PSUM"))
    bcpsum = ctx.enter_context(tc.tile_pool(name="bcpsum", bufs=2, space="PSUM"))
    tpsum = ctx.enter_context(tc.tile_pool(name="tpsum", bufs=1, space="PSUM"))
    small = ctx.enter_context(tc.tile_pool(name="small", bufs=2))

    # ---- constants (no input deps; scheduled early) ---------------------
    ident = singles.tile([128, 128], f32)
    make_identity(nc, ident)
    ones = singles.tile([1, 128], f32)
    nc.vector.memset(ones, 1.0)

    # ---- load hidden^T (tiny) -------------------------------------------
    hid_sb = singles.tile([128, NHC * BPC], f32)
    nc.sync.dma_start(out=hid_sb, in_=hidT)

    # ---- v = W^T h : PE matmul, contraction over h ----------------------
    # out[m=b, n=k] = sum_h hidT[h, b] * W[h, k], accumulated over NHC chunks
    v_ps = [
        vpsum.tile([BPC, KC], f32, name=f"v_ps{i}", tag=f"v_ps{i}")
        for i in range(NKC)
    ]
    for c in range(NHC):
        w_sb = wpool.tile([128, K], f32)
        nc.sync.dma_start(out=w_sb, in_=w[c * HC:(c + 1) * HC, :])
        for kc in range(NKC):
            nc.tensor.matmul(
                v_ps[kc][:, :],
                lhsT=hid_sb[:, c * BPC:(c + 1) * BPC],
                rhs=w_sb[:, kc * KC:(kc + 1) * KC],
                start=(c == 0),
                stop=(c == NHC - 1),
            )

    v_sb = singles.tile([BPC, K], f32)
    for kc in range(NKC):
        nc.scalar.copy(out=v_sb[:, kc * KC:(kc + 1) * KC], in_=v_ps[kc][:, :])

    # ---- broadcast v to all 128 partitions via K=1 ones-matmul on PE ----
    # Each v row is first moved to partition 0 (matmul rhs needs base 0).
    v_bc = singles.tile([128, BPC, K], f32)
    for b in range(BPC):
        v_row = small.tile([1, K], f32, name="v_row", tag="v_row")
        nc.sync.dma_start(out=v_row, in_=v_sb[b:b + 1, :])
        for kc in range(NKC):
            bc_ps = bcpsum.tile([128, KC], f32, name="bc_ps", tag="bc_ps")
            nc.tensor.matmul(
                bc_ps[:, :],
                lhsT=ones,
                rhs=v_row[0:1, kc * KC:(kc + 1) * KC],
                start=True,
                stop=True,
            )
            eng = nc.vector if (b * NKC + kc) % 2 == 0 else nc.scalar
            if eng is nc.vector:
                eng.tensor_copy(v_bc[:, b, kc * KC:(kc + 1) * KC], bc_ps[:, :])
            else:
                eng.copy(out=v_bc[:, b, kc * KC:(kc + 1) * KC], in_=bc_ps[:, :])

    # ---- main sweep: scores[s,b] = enc[s,b,:] . v_b ---------------------
    # DVE does the elementwise multiply; ScalarE (activation Copy with
    # accum_out) does the free-dim sum, so the two passes run on separate
    # engines and both stay under the DMA streaming time.
    scores = singles.tile([128, BPC, NST], f32)
    NBG = BPC // BGRP
    for st in range(NST):
        for g in range(NBG):
            enc_sb = encpool.tile([128, BGRP, K], f32)
            eng = nc.sync if (st * NBG + g) % 2 == 0 else nc.scalar
            eng.dma_start(
                out=enc_sb,
                in_=enc[st * ST:(st + 1) * ST, g * BGRP:(g + 1) * BGRP, :],
            )
            for bi in range(BGRP):
                b = g * BGRP + bi
                prod = prodpool.tile([128, K], f32, name="prod", tag="prod")
                nc.vector.tensor_mul(prod, enc_sb[:, bi, :], v_bc[:, b, :])
                nc.scalar.activation(
                    out=prod,
                    in_=prod,
                    func=mybir.ActivationFunctionType.Copy,
                    bias=0.0,
                    scale=1.0,
                    accum_out=scores[:, b, st:st + 1],
                )

    # ---- softmax over s (s is split: 128 partitions x NST cols) ---------
    m = small.tile([128, BPC], f32)
    nc.vector.tensor_reduce(
        out=m, in_=scores, axis=mybir.AxisListType.X, op=mybir.AluOpType.max
    )
    mm = small.tile([128, BPC], f32)
    nc.gpsimd.partition_all_reduce(mm, m, 128, ReduceOp.max)
    nm = small.tile([128, BPC], f32)
    nc.vector.tensor_scalar_mul(nm, mm, -1.0)

    e = singles.tile([128, BPC, NST], f32)
    r = small.tile([128, BPC], f32)
    for b in range(BPC):
        nc.scalar.activation(
            out=e[:, b, :],
            in_=scores[:, b, :],
            func=mybir.ActivationFunctionType.Exp,
            bias=nm[:, b:b + 1],
            scale=1.0,
            accum_out=r[:, b:b + 1],
        )
    rs = small.tile([128, BPC], f32)
    nc.gpsimd.partition_all_reduce(rs, r, 128, ReduceOp.add)
    inv = small.tile([128, BPC], f32)
    nc.vector.reciprocal(inv, rs)

    attn = singles.tile([128, BPC * NST], f32)
    attn3 = attn.rearrange("p (b t) -> p b t", b=BPC)
    for b in range(BPC):
        nc.vector.tensor_scalar_mul(attn3[:, b, :], e[:, b, :], inv[:, b:b + 1])

    # ---- transpose [128, 64] -> [64, 128] and store ---------------------
    att_ps = tpsum.tile([BPC * NST, 128], f32)
    nc.tensor.transpose(att_ps[:, :], attn[:, :], ident[:, :])
    att_sb = small.tile([BPC * NST, 128], f32)
    nc.scalar.copy(out=att_sb, in_=att_ps[:, :])
    nc.sync.dma_start(
        out=out.rearrange("b (t s) -> b t s", s=ST), in_=att_sb
    )


def _build():
    if "nc" in _CACHE:
        return _CACHE["nc"]
    from contextlib import ExitStack

    import concourse.bacc as bacc
    import concourse.tile as tile
    from concourse import mybir

    nc = bacc.Bacc(
        "TRN2", target_bir_lowering=False, debug=False, num_devices=N_CORES
    )
    enc_d = nc.dram_tensor("enc", [S, BPC, K], mybir.dt.float32, kind="ExternalInput")
    hid_d = nc.dram_tensor(
        "hidT", [128, NHC * BPC], mybir.dt.float32, kind="ExternalInput"
    )
    w_d = nc.dram_tensor("w", [H, K], mybir.dt.float32, kind="ExternalInput")
    out_d = nc.dram_tensor(
        "attn_out", [BPC, S], mybir.dt.float32, kind="ExternalOutput"
    )

    with tile.TileContext(nc) as tc:
        with ExitStack() as ctx:
            _emit(ctx, tc, enc_d.ap(), hid_d.ap(), w_d.ap(), out_d.ap())
    nc.compile()
    _CACHE["nc"] = nc
    return nc


def _make_in_maps(hidden, encoder_outputs, W):
    in_maps = []
    w = np.ascontiguousarray(W, dtype=np.float32)
    for i in range(N_CORES):
        b0 = i * BPC
        # hidT layout [p][c][b] with h = c*128 + p
        hid = hidden[0, b0:b0 + BPC, :]                    # [BPC, H]
        hidT = np.ascontiguousarray(
            hid.T.reshape(NHC, 128, BPC).transpose(1, 0, 2).reshape(128, NHC * BPC),
            dtype=np.float32,
        )
        enc = np.ascontiguousarray(
            encoder_outputs[:, b0:b0 + BPC, :], dtype=np.float32
        )
        in_maps.append({"enc": enc, "hidT": hidT, "w": w})
    return in_maps


def kernel(hidden, encoder_outputs, W, b):
    from concourse import bass_utils

    nc = _build()
    in_maps = _make_in_maps(
        np.asarray(hidden), np.asarray(encoder_outputs), np.asarray(W)
    )
    res = bass_utils.run_bass_kernel_spmd(
        nc, in_maps, core_ids=list(range(N_CORES))
    )
    out = np.concatenate(
        [res.results[i]["attn_out"] for i in range(N_CORES)], axis=0
    )  # [B, S]
    return out[:, None, :].astype(np.float32)
